# revision 35
# baseline (speedup 1.0000x reference)
"""Trainium2 Bass kernel for scatter-memory GRU update.

reference semantics (single-device jax, CPU):
    current = memory[node_ids]                 # [B, H] gather
    h_new   = GRUCell(messages, current)       # [B, H]
    out     = memory.at[node_ids].set(h_new)   # last occurrence wins

Strategy (8 NeuronCores):
  * Shard the 1M-row memory table row-wise: core c owns rows
    [c*125000, (c+1)*125000), split into 4 sub-tables of 31250 rows so
    local row indices fit int16 (dma_gather/dma_scatter_add requirement).
  * Host routes (node_id, message) pairs to the owning (core, sub-table)
    bucket, deduping to the last occurrence per id (matches jax-CPU
    scatter semantics).
  * The output DRAM tensors are donated jax buffers PRE-LOADED with the
    memory shard (the same donation mechanism run_bass_via_pjrt relies on
    with zero buffers; unwritten rows keep their donated content), so the
    kernel never copies or even reads the f32 table except for the rows
    it updates.
  * A bf16 shadow of the shard (host-cast) is gathered TRANSPOSED
    (dma_gather(transpose=True)) so h arrives in [feature, item] layout
    ready to be the moving operand of the GRU matmuls - no PE transposes
    or PSUM evacuation for h.
  * GRU runs in [feature, item] layout: all-bf16 matmuls (1 PE
    cycle/row), sigmoid/tanh on ACT, elementwise on DVE with the b_hn
    bias fused via scalar_tensor_tensor. delta = (n - h)*(1 - z) is
    transposed back (PE) and dma_scatter_add'ed onto the preloaded rows
    (mem + delta == h_new).
  * Padding slots scatter into a dummy 31251st row that the host drops.
"""

import numpy as np

NUM_NODES = 1_000_000
MEM_DIM = 128
N_CORES = 8
N_SUB = 4
ROWS_CORE = NUM_NODES // N_CORES       # 125000
ROWS_SUB = ROWS_CORE // N_SUB          # 31250
N_BUCKETS = N_CORES * N_SUB            # 32
CHUNK_ITEMS = 512                      # items per compute chunk (PSUM bank)
N_GS_CHUNKS = 4                        # gather/scatter calls per sub-table
N_SWDGE_QUEUES = 1                     # tile sem lanes assume a single queue


def _host_prep(node_ids, messages):
    ids = np.ascontiguousarray(np.asarray(node_ids).astype(np.int64))
    msgs = np.ascontiguousarray(np.asarray(messages).astype(np.float32))
    B = len(ids)
    # unique with LAST occurrence winning (jax-CPU .at[].set semantics)
    u, ri = np.unique(ids[::-1], return_index=True)
    win_pos = B - 1 - ri
    bounds = np.searchsorted(u, np.arange(N_BUCKETS + 1) * ROWS_SUB)
    counts = np.diff(bounds)
    cap = max(512, int(np.ceil(counts.max() / CHUNK_ITEMS) * CHUNK_ITEMS))
    S = cap // 16

    per_core = []
    for c in range(N_CORES):
        gidx = np.zeros((16, N_SUB * S), np.int16)
        sidx = np.zeros((16, N_SUB * S), np.int16)
        msgsT = np.zeros((MEM_DIM, N_SUB * cap), np.float32)
        for k in range(N_SUB):
            b = c * N_SUB + k
            lo, hi = bounds[b], bounds[b + 1]
            n = hi - lo
            loc = (u[lo:hi] - b * ROWS_SUB).astype(np.int16)
            g = np.zeros(cap, np.int16)            # gather pad -> row 0
            s = np.full(cap, ROWS_SUB, np.int16)   # scatter pad -> dummy row
            g[:n] = loc
            s[:n] = loc
            gidx[:, k * S:(k + 1) * S] = g.reshape(S, 16).T
            sidx[:, k * S:(k + 1) * S] = s.reshape(S, 16).T
            msgsT[:, k * cap:k * cap + n] = msgs[win_pos[lo:hi]].T
        per_core.append({
            "gidx": np.ascontiguousarray(np.tile(gidx, (8, 1))),
            "sidx": np.ascontiguousarray(np.tile(sidx, (8, 1))),
            "msgsT": msgsT,
        })
    return per_core, cap


def _build_program(cap, repeats=1, ablate=(), n_gs_chunks=N_GS_CHUNKS,
                   big_chunks=False, queue_mode="split"):
    import concourse.bass as bass
    import concourse.bacc as bacc
    import concourse.mybir as mybir
    import concourse.tile as tile
    from concourse.masks import make_identity
    from concourse.tile_rust import add_dep_helper

    f32 = mybir.dt.float32
    bf16 = mybir.dt.bfloat16
    i16 = mybir.dt.int16
    AF = mybir.ActivationFunctionType
    ALU = mybir.AluOpType
    S = cap // 16
    T = cap // 128
    if big_chunks:
        W = 1024
        chunk_plan = [1024] * (cap // 1024)
        if cap % 1024:
            chunk_plan.append(512)
    else:
        W = CHUNK_ITEMS
        chunk_plan = [CHUNK_ITEMS] * (cap // CHUNK_ITEMS)

    n_queues = {"single": 1, "split": 2, "split3": 4, "spread": 4}[queue_mode]

    def gq_num(q):
        return 0 if queue_mode != "spread" else q % 4

    def sq_num(q):
        if queue_mode == "single":
            return 0
        if queue_mode == "split":
            return 1
        if queue_mode == "split3":
            return 1 + (q % 3)
        return q % 4

    nc = bacc.Bacc(None, target_bir_lowering=False,
                   num_swdge_queues=n_queues)
    memB = [nc.declare_dram_parameter(f"memB{k}", [ROWS_SUB, MEM_DIM], bf16,
                                      isOutput=False) for k in range(N_SUB)]
    msgsT_d = nc.declare_dram_parameter("msgsT", [MEM_DIM, N_SUB * cap], bf16,
                                        isOutput=False)
    gidx_d = nc.declare_dram_parameter("gidx", [128, N_SUB * S], i16,
                                       isOutput=False)
    sidx_d = nc.declare_dram_parameter("sidx", [128, N_SUB * S], i16,
                                       isOutput=False)
    wb_d = nc.declare_dram_parameter("wb", [MEM_DIM, 6 * MEM_DIM], bf16,
                                     isOutput=False)
    bias_d = nc.declare_dram_parameter("bias", [MEM_DIM, 4], f32,
                                       isOutput=False)
    # Donated jax buffers preloaded with the memory shard: rows the kernel
    # never writes keep the memory content, so no table copy is needed.
    out = [nc.declare_dram_parameter(f"out{k}", [ROWS_SUB + 1, MEM_DIM], f32,
                                     isOutput=True) for k in range(N_SUB)]

    with tile.TileContext(nc) as tc:
        with (
            tc.tile_pool(name="const", bufs=1) as cpool,
            tc.tile_pool(name="hT", bufs=2) as hpool,
            tc.tile_pool(name="d", bufs=2) as dpool,
            tc.tile_pool(name="work", bufs=2) as wpool,
            tc.tile_pool(name="xc", bufs=3) as xcpool,
            tc.tile_pool(name="ps", bufs=1 if big_chunks else 2,
                         space="PSUM") as ppool,
            tc.tile_pool(name="px", bufs=1, space="PSUM") as pxpool,
        ):
            wb_sb = cpool.tile([128, 6 * MEM_DIM], bf16)
            b_sb = cpool.tile([128, 4], f32)
            gidx_sb = cpool.tile([128, N_SUB * S], i16)
            sidx_sb = cpool.tile([128, N_SUB * S], i16)
            ident = cpool.tile([128, 128], bf16)
            nc.sync.dma_start(out=wb_sb[:], in_=wb_d[:])
            nc.sync.dma_start(out=b_sb[:], in_=bias_d[:])
            nc.sync.dma_start(out=gidx_sb[:], in_=gidx_d[:])
            nc.sync.dma_start(out=sidx_sb[:], in_=sidx_d[:])
            make_identity(nc, ident[:])

            last_scatters = {k: [] for k in range(N_SUB)}
            for rep in range(repeats):
              for k in range(N_SUB):
                # ---- gather current rows (bf16): h[i] = memB[gidx[i]] ----
                h_sb = hpool.tile([128, T * MEM_DIM], bf16, tag="h")
                h3 = h_sb[:].rearrange("p (t d) -> p t d", d=MEM_DIM)
                gq = cap // n_gs_chunks
                gather_insts = []
                if "gather" not in ablate:
                    for q in range(n_gs_chunks):
                        gi = nc.gpsimd.dma_gather(
                            out_ap=h3[:, q * (gq // 128):(q + 1) * (gq // 128), :],
                            in_ap=memB[k][:, :],
                            idxs_ap=gidx_sb[:, k * S + q * (gq // 16):
                                            k * S + (q + 1) * (gq // 16)],
                            num_idxs=gq,
                            num_idxs_reg=gq,
                            elem_size=MEM_DIM,
                            single_packet=False,
                            queue_num=gq_num(q),
                        )
                        for sc_prev in last_scatters[k]:
                            add_dep_helper(gi.ins, sc_prev,
                                           reason="gather after prev scatter")
                        gather_insts.append(gi.ins)
                elif "compute" not in ablate:
                    nc.gpsimd.memset(h_sb[:], 0.0)

                # item-major delta rows for the scatter
                d_sb = dpool.tile([128, T * MEM_DIM], f32, tag="d")
                d3 = d_sb[:].rearrange("p (t d) -> p t d", d=MEM_DIM)
                if "compute" in ablate and "scatter" not in ablate:
                    nc.gpsimd.memset(d_sb[:], 0.0)

                # ---- GRU in [feature, item] layout, chunked ----
                i0 = 0
                for csize in (chunk_plan if "compute" not in ablate else []):
                    xc = xcpool.tile([128, W], bf16, tag="xc")
                    nc.sync.dma_start(
                        out=xc[:, :csize],
                        in_=msgsT_d[:, k * cap + i0:k * cap + i0 + csize])

                    # transpose h chunk -> hT [feat, item] (bf16, 1 cyc/row)
                    psum_t = pxpool.tile([128, W], bf16, tag="pt")
                    for t in range(csize // 128):
                        nc.tensor.transpose(
                            out=psum_t[:, t * 128:(t + 1) * 128],
                            in_=h_sb[:, i0 + t * 128:i0 + (t + 1) * 128],
                            identity=ident[:])
                    hT_t = wpool.tile([128, W], bf16, tag="hT")
                    nc.vector.tensor_copy(hT_t[:, :csize], psum_t[:, :csize])
                    hT = hT_t[:, :csize]

                    def gate_mm(psum, wx_col, wh_col):
                        # one matmul group per 512-wide PSUM-bank half
                        for s in range(0, csize, 512):
                            e = min(s + 512, csize)
                            if wx_col is not None:
                                nc.tensor.matmul(
                                    psum[:, s:e],
                                    lhsT=wb_sb[:, wx_col:wx_col + 128],
                                    rhs=xc[:, s:e], start=True,
                                    stop=wh_col is None)
                            if wh_col is not None:
                                nc.tensor.matmul(
                                    psum[:, s:e],
                                    lhsT=wb_sb[:, wh_col:wh_col + 128],
                                    rhs=hT_t[:, s:e], start=wx_col is None,
                                    stop=True)

                    # r gate
                    psum_r = ppool.tile([128, W], f32, tag="pr")
                    gate_mm(psum_r, 0, 384)
                    r = wpool.tile([128, W], bf16, tag="r")
                    nc.scalar.activation(r[:, :csize], psum_r[:, :csize],
                                         AF.Sigmoid, bias=b_sb[:, 0:1])

                    # n gate, h side: t1 = (psum_hn + b_hn) * r
                    psum_hn = ppool.tile([128, W], f32, tag="phn")
                    gate_mm(psum_hn, None, 640)
                    t1 = wpool.tile([128, W], f32, tag="t1")
                    nc.vector.scalar_tensor_tensor(
                        t1[:, :csize], psum_hn[:, :csize], b_sb[:, 3:4],
                        r[:, :csize], op0=ALU.add, op1=ALU.mult)

                    # z gate (as zp = 1 - z via sigmoid(-x))
                    psum_z = ppool.tile([128, W], f32, tag="pz")
                    gate_mm(psum_z, 128, 512)
                    zp = wpool.tile([128, W], bf16, tag="zp")
                    nc.scalar.activation(zp[:, :csize], psum_z[:, :csize],
                                         AF.Sigmoid, bias=b_sb[:, 1:2],
                                         scale=-1.0)

                    # n gate, x side (alternates with pz in its pool slots)
                    psum_gn = ppool.tile([128, W], f32, tag="pz")
                    gate_mm(psum_gn, 256, None)
                    t2 = wpool.tile([128, W], f32, tag="t2")
                    nc.vector.tensor_add(t2[:, :csize], t1[:, :csize],
                                         psum_gn[:, :csize])
                    n_t = wpool.tile([128, W], bf16, tag="nt")
                    nc.scalar.activation(n_t[:, :csize], t2[:, :csize],
                                         AF.Tanh, bias=b_sb[:, 2:3])

                    # delta = (n - h) * (1 - z), then back to [item, feat]
                    m = wpool.tile([128, W], bf16, tag="m")
                    nc.vector.tensor_sub(m[:, :csize], n_t[:, :csize], hT)
                    dT = wpool.tile([128, W], bf16, tag="dT")
                    nc.vector.tensor_mul(dT[:, :csize], m[:, :csize],
                                         zp[:, :csize])

                    psum_d = pxpool.tile([128, W], bf16, tag="pd")
                    for t in range(csize // 128):
                        nc.tensor.transpose(
                            out=psum_d[:, t * 128:(t + 1) * 128],
                            in_=dT[:, t * 128:(t + 1) * 128],
                            identity=ident[:])
                    nc.scalar.activation(d_sb[:, i0:i0 + csize],
                                         psum_d[:, :csize], AF.Copy)
                    i0 += csize

                # ---- scatter-add deltas onto the preloaded memory rows ----
                if "scatter" not in ablate:
                    last_scatters[k] = []
                    for q in range(n_gs_chunks):
                        sc = nc.gpsimd.dma_scatter_add(
                            out[k][:, :],
                            d3[:, q * (gq // 128):(q + 1) * (gq // 128), :],
                            sidx_sb[:, k * S + q * (gq // 16):
                                    k * S + (q + 1) * (gq // 16)],
                            gq,
                            gq,
                            MEM_DIM,
                            single_packet=False,
                            queue_num=sq_num(q),
                        )
                        last_scatters[k].append(sc.ins)
    nc.compile()
    return nc


def _make_in_maps(inputs, per_core):
    import ml_dtypes
    memory = np.ascontiguousarray(np.asarray(inputs["memory"],
                                             dtype=np.float32))
    memB = memory.astype(ml_dtypes.bfloat16)
    W_ih = np.asarray(inputs["W_ih"], dtype=np.float32)
    W_hh = np.asarray(inputs["W_hh"], dtype=np.float32)
    b_ih = np.asarray(inputs["b_ih"], dtype=np.float32)
    b_hh = np.asarray(inputs["b_hh"], dtype=np.float32)

    wb = np.ascontiguousarray(
        np.concatenate([W_ih.T, W_hh.T], axis=1)).astype(
            ml_dtypes.bfloat16)                             # [128, 768]
    bias = np.stack([
        b_ih[0:128] + b_hh[0:128],
        -(b_ih[128:256] + b_hh[128:256]),
        b_ih[256:384],
        b_hh[256:384],
    ], axis=1).astype(np.float32)                           # [128, 4]

    in_maps = []
    for c in range(N_CORES):
        m = {
            "msgsT": per_core[c]["msgsT"].astype(ml_dtypes.bfloat16),
            "gidx": per_core[c]["gidx"],
            "sidx": per_core[c]["sidx"],
            "wb": wb,
            "bias": bias,
        }
        for k in range(N_SUB):
            b = c * N_SUB + k
            m[f"memB{k}"] = memB[b * ROWS_SUB:(b + 1) * ROWS_SUB]
        in_maps.append(m)
    return in_maps


def _make_preloads(memory):
    """Per-core donated initial contents for out0..out3: the memory shard
    plus one dummy row (index ROWS_SUB) that padded scatter slots hit."""
    memory = np.ascontiguousarray(np.asarray(memory, dtype=np.float32))
    preloads = []
    for c in range(N_CORES):
        m = {}
        for k in range(N_SUB):
            b = c * N_SUB + k
            buf = np.zeros((ROWS_SUB + 1, MEM_DIM), np.float32)
            buf[:ROWS_SUB] = memory[b * ROWS_SUB:(b + 1) * ROWS_SUB]
            m[f"out{k}"] = buf
        preloads.append(m)
    return preloads


def _make_runner(nc, n_cores=N_CORES):
    """Jitted SPMD callable that donates caller-supplied output buffers
    (so their preloaded content survives for rows the kernel skips)."""
    import jax
    import concourse.mybir as mybir
    from concourse import bass2jax
    from jax.sharding import Mesh, PartitionSpec
    from jax.experimental.shard_map import shard_map

    bass2jax.install_neuronx_cc_hook()
    partition_name = (nc.partition_id_tensor.name
                      if nc.partition_id_tensor else None)
    in_names, out_names, out_avals = [], [], []
    for alloc in nc.m.functions[0].allocations:
        if not isinstance(alloc, mybir.MemoryLocationSet):
            continue
        name = alloc.memorylocations[0].name
        if alloc.kind == "ExternalInput":
            if name != partition_name:
                in_names.append(name)
        elif alloc.kind == "ExternalOutput":
            out_names.append(name)
            out_avals.append(jax.core.ShapedArray(
                tuple(alloc.tensor_shape), mybir.dt.np(alloc.dtype)))
    n_params = len(in_names)
    all_in_names = list(in_names) + out_names
    if partition_name is not None:
        all_in_names.append(partition_name)
    donate = tuple(range(n_params, n_params + len(out_names)))

    def _body(*args):
        operands = list(args)
        if partition_name is not None:
            operands.append(bass2jax.partition_id_tensor())
        outs = bass2jax._bass_exec_p.bind(
            *operands, out_avals=tuple(out_avals),
            in_names=tuple(all_in_names), out_names=tuple(out_names),
            lowering_input_output_aliases=(),
            sim_require_finite=True, sim_require_nnan=True, nc=nc)
        return tuple(outs)

    devices = jax.devices()[:n_cores]
    mesh = Mesh(np.asarray(devices), ("core",))
    spec = PartitionSpec("core")
    sharded = jax.jit(
        shard_map(_body, mesh=mesh,
                  in_specs=(spec,) * (n_params + len(out_names)),
                  out_specs=(spec,) * len(out_names), check_rep=False),
        donate_argnums=donate, keep_unused=True)
    return sharded, in_names, out_names, out_avals


def _run_with_preload(nc, in_maps, preloads, n_cores=N_CORES):
    import jax
    sharded, in_names, out_names, out_avals = _make_runner(nc, n_cores)
    concat_in = [np.concatenate([np.asarray(m[nm]) for m in in_maps], axis=0)
                 for nm in in_names]
    concat_pre = [np.concatenate([np.asarray(p[nm]) for p in preloads],
                                 axis=0) for nm in out_names]
    outs = sharded(*concat_in, *concat_pre)
    res = []
    for c in range(n_cores):
        res.append({nm: np.asarray(outs[i]).reshape(
            n_cores, *out_avals[i].shape)[c]
            for i, nm in enumerate(out_names)})
    return res


def _run(inputs):
    per_core, cap = _host_prep(inputs["node_ids"], inputs["messages"])
    in_maps = _make_in_maps(inputs, per_core)
    preloads = _make_preloads(inputs["memory"])
    nc = _build_program(cap)
    res = _run_with_preload(nc, in_maps, preloads)

    outp = np.empty((NUM_NODES, MEM_DIM), np.float32)
    for c in range(N_CORES):
        for k in range(N_SUB):
            b = c * N_SUB + k
            outp[b * ROWS_SUB:(b + 1) * ROWS_SUB] = \
                res[c][f"out{k}"][:ROWS_SUB]
    return outp


def kernel(**inputs):
    return _run(inputs)
